# revision 56
# baseline (speedup 1.0000x reference)
"""Trainium2 Bass kernel: 16-head MHA (S=2048, D=1024, Dk=Dv=64) on 8 NeuronCores.

Sharding: tensor-parallel over heads - 2 heads per core. Each core projects
Q/K/V for its 2 heads, computes scores in transposed layout S^T[t, s] =
K_h Q_h^T, exponentiates with the 1/sqrt(64) scale fused in, and accumulates
heads^T = V_aug^T @ exp(S^T) with a ones-column so the softmax denominator
falls out of the same matmul (PSUM row 64).

Differences vs the naive schedule, all aimed at keeping the PE tensor engine
busy at full clock end to end:
  - PE warm-up matmuls run during the initial DMA wait so the p-state ramps
    to full frequency before the first real matmul.
  - Embeddings stream in four 1MB chunks (host-prearranged to contiguous
    per-chunk layout) and QKV compute starts as soon as chunk 0 lands.
  - The softmax exp is split across engines: most tiles use the exact Exp on
    the Scalar/ACT engine (output scaled by the mean Schraudolph factor so
    the two tile families agree), the rest use a Schraudolph-style bit-trick
    exp on the Vector engine (affine op + fp32->int16 convert, int16 bits
    reinterpreted as bf16).
  - The V projection bias is folded into the per-core output bias on the
    host (softmax weights sum to one, so +bv passes through attention and
    lands in Wo^T bv).
  - Softmax normalization: ACT evacuates the AV psum, DVE takes the
    reciprocal of the denominator row, GpSimd broadcasts it and applies the
    multiply (all off the critical engines).
  - The sh=0 output projection is scheduled between attention passes 2 and 3
    and filler matmuls run while the last softmax normalization completes,
    so the PE never idles long enough to drop its p-state before the final
    output projection.
  - Output partials ship as bf16 (summed in fp32 on the host), halving the
    output drain, and all output-DMA triggers ride the otherwise-idle sync
    sequencer.

Matmuls run in bf16 with fp32 PSUM accumulation.
"""

import math

import numpy as np

import concourse.tile as tile_mod
from concourse import bacc, mybir
from concourse.bass_utils import run_bass_kernel_spmd
from concourse.vector_clock import ScopedClock, VectorClock

F32 = mybir.dt.float32
BF16 = mybir.dt.bfloat16
I16 = mybir.dt.int16

S, D, H, DK = 2048, 1024, 16, 64
P = 128
NCORES = 8

# Schraudolph bf16 exp: u = bitcast_bf16(int16(s * SCH_A + SCH_B)) ~= exp(s/8)
SCH_A = 0.125 * 128.0 / math.log(2.0)
SCH_B = 127.0 * 128.0
# The bit-trick overestimates exp by x1.0418 on average; scale the exact-exp
# tiles by the same factor (bias adds ln inside the exponent) so both tile
# families are consistently biased and the bias cancels in the softmax.
ACT_BIAS = 0.0399
# Per-pass split of the 16 t-tiles: 8 on ACT (exact), 8 on DVE (bit trick).
# tb 0/1 go to DVE so a new pass never waits on ACT finishing the previous
# pass's evac+ln.
ACT_TBS = frozenset([1, 2, 4, 6, 8, 10, 12, 14])

N_WARMUP = 12
N_FILLER = 14


def _patched_drain_and_barrier(self, tick_clock, wait_clock):
    """This container's walrus build caps CTRL-type instructions at one sem
    wait, but Tile's exit drain carries one wait per outstanding proc. Emit
    one Drain per outstanding proc instead, each with a single wait."""
    gc = tick_clock.global_clock
    vec = list(gc)
    for i, t in enumerate(vec):
        if t <= 0:
            continue
        pv = [0] * len(vec)
        pv[i] = t
        d = self.nc.sync.drain()
        wait_clock.add_sem_waits(d.ins, ScopedClock({None: VectorClock(pv)}))

    self.nc.all_engine_barrier()
    assert self.sems is not None
    popped = self.nc._tile_sem_poison_stack.pop()
    assert popped is self._sem_poison
    self.nc.clear_and_free_semaphores(list(self.sems.allocated().values()))
    self.nc.all_engine_barrier()


tile_mod.TileContext._drain_and_barrier = _patched_drain_and_barrier


def _pin_act_tables():
    """Pin ACT-table selection to natural_log_exp_and_others (covers Exp,
    Ln, Copy, Identity) so the per-pass Exp<->Ln mix doesn't thrash
    ACT_TABLE_LOADs. Other sets are emptied but keep their positions so the
    emitted act_func_set_id still indexes act_info.json correctly."""
    import concourse.bacc as bacc_mod

    orig = bacc_mod.get_activation_tables
    keep = "natural_log_exp_and_others"

    def pinned(arch, _orig=orig, _keep=keep):
        t = dict(_orig(arch))
        if _keep not in t:
            return t
        return {n: (f if n == _keep else set()) for n, f in t.items()}

    bacc_mod.get_activation_tables = pinned


_pin_act_tables()


def _build_nc():
    from contextlib import ExitStack

    tile = tile_mod
    nc = bacc.Bacc(None)

    # et: embeddings^T prearranged on host as [sc, pi, dc*512+sl] so each
    # 1MB s-chunk is one contiguous DMA with 8KB runs per partition.
    # wqkv: host-prearranged as [grp, pi, dc*128+c] (grp = q/k/v) so each
    # projection's weights are one contiguous transfer.
    # out: [pi, sh*8192 + blk*1024 + sl] so each s-half ships as one DMA
    # with 16KB contiguous runs per partition (128 descriptors instead of
    # 2048 1KB ones).
    et = nc.declare_dram_parameter("et", [4, P, 4096], BF16, isOutput=False)
    wqkv = nc.declare_dram_parameter("wqkv", [P, 3072], BF16, isOutput=False)
    bqk = nc.declare_dram_parameter("bqk", [P, 2], F32, isOutput=False)
    bo = nc.declare_dram_parameter("bo", [P, 8], F32, isOutput=False)
    wo = nc.declare_dram_parameter("wo", [P, D], BF16, isOutput=False)
    out = nc.declare_dram_parameter("out", [P, 16384], BF16, isOutput=True)

    with tile.TileContext(nc) as tc, ExitStack() as ctx:
        consts = ctx.enter_context(tc.tile_pool(name="consts", bufs=1))
        qkv = ctx.enter_context(tc.tile_pool(name="qkv", bufs=1))
        utp = ctx.enter_context(tc.tile_pool(name="ut", bufs=3))
        headsp = ctx.enter_context(tc.tile_pool(name="heads", bufs=2))
        normp = ctx.enter_context(tc.tile_pool(name="norm", bufs=4))
        outp = ctx.enter_context(tc.tile_pool(name="outp", bufs=3))
        psum = ctx.enter_context(tc.tile_pool(name="psum", bufs=1, space="PSUM"))

        # ---- input DMAs ----------------------------------------------------
        # Small consts first, then wqkv split across both HWDGE rings, then
        # the four et chunks (each split across rings), wo last.
        bqk_sb = consts.tile([P, 2], F32)
        nc.sync.dma_start(bqk_sb[:], bqk[:])
        bo_c = consts.tile([P, 8], F32)
        nc.scalar.dma_start(bo_c[:], bo[:])
        wqkv_sb = consts.tile([P, 3, 8, P], BF16)  # [pi, grp, dc, c]
        nc.sync.dma_start(wqkv_sb[:, 0], wqkv[:, 0:1024])
        nc.scalar.dma_start(wqkv_sb[:, 1:3], wqkv[:, 1024:3072])
        et_sc = []
        for sc in range(4):
            t = consts.tile([P, 8, 512], BF16, name=f"et{sc}")
            et_sc.append(t)
            eng = nc.scalar if sc % 2 else nc.sync
            eng.dma_start(t[:], et[sc])
        wo_sb = consts.tile([P, D], BF16)
        nc.scalar.dma_start(wo_sb[:], wo[:])

        # ---- SBUF compute tiles -------------------------------------------
        warm = consts.tile([P, 512], BF16)
        nc.vector.memset(warm[:], 0.25)
        act_bias = consts.tile([P, 1], F32)
        nc.vector.memset(act_bias[:], ACT_BIAS)
        qt_sb = qkv.tile([P, S], BF16)
        kt_sb = qkv.tile([P, S], BF16)
        # vaug[t-part, tb, head, 0:64]=v, [*, 64]=1 (softmax denominator row)
        vaug_sb = qkv.tile([P, 16, 2, 65], BF16)
        nc.vector.memset(vaug_sb[:, :, :, 64:65], 1.0)

        # PSUM: st [128,1024]x2 (4 banks), av [65,1024]x1 (2), op [128,512]x2
        # (2) = 8 banks. QKV-phase psums rotate over the same slots.
        mm_bufs = {"st": 2, "av": 1, "op": 2}
        rot_state = [0]
        rot_cycle = ["op", "op", "st", "st", "av"]

        def next_tag():
            t = rot_cycle[rot_state[0] % 5]
            rot_state[0] += 1
            return t

        # ---- PE warm-up ----------------------------------------------------
        # Dead matmuls during the DMA wait ramp the PE p-state to full clock.
        for i in range(N_WARMUP):
            wps = psum.tile([P, 512], F32, tag="op", bufs=2, name=f"warm{i}")
            nc.tensor.matmul(
                wps[:], warm[:, 0:P], warm[:], start=True, stop=True
            )

        # ---- QKV projections, one et chunk at a time -----------------------
        for sc in range(4):
            s0 = sc * 512
            for which, dst in ((0, qt_sb), (1, kt_sb)):
                tg = next_tag()
                ps = psum.tile(
                    [P, 512], F32, tag=tg, bufs=mm_bufs[tg], name=f"qk{sc}{which}"
                )
                for dc in range(8):
                    nc.tensor.matmul(
                        ps[:],
                        wqkv_sb[:, which, dc, :],
                        et_sc[sc][:, dc, :],
                        start=(dc == 0),
                        stop=(dc == 7),
                    )
                nc.vector.tensor_scalar_add(
                    dst[:, s0 : s0 + 512], ps[:], bqk_sb[:, which : which + 1]
                )
            for tl in range(4):
                tb = sc * 4 + tl
                tg = next_tag()
                ps = psum.tile(
                    [P, P], F32, tag=tg, bufs=mm_bufs[tg], name=f"v{tb}"
                )
                for dc in range(8):
                    nc.tensor.matmul(
                        ps[:],
                        et_sc[sc][:, dc, tl * P : tl * P + P],
                        wqkv_sb[:, 2, dc, :],
                        start=(dc == 0),
                        stop=(dc == 7),
                    )
                # v psum cols [0:64]=head0, [64:128]=head1 -> vaug (no bias:
                # bv is folded into bo on the host)
                nc.scalar.activation(
                    vaug_sb[:, tb, :, 0:64],
                    ps[:].rearrange("p (hh v) -> p hh v", hh=2),
                    mybir.ActivationFunctionType.Copy,
                )

        # Per-s-half output accumulators: all 16 units' results collect here
        # and ship as ONE 2MB DMA with 16KB contiguous runs per partition.
        ot_all = [
            outp.tile([P, 8, 1024], BF16, tag=f"ot{sh}", bufs=1, name=f"ot{sh}")
            for sh in range(2)
        ]

        def emit_op_unit(sh, heads_sb, blk, ch, tg="op", bufs=2):
            # out^T[c, s] = wo_rows.T @ heads^T (+ bo as per-partition scalar)
            c0 = blk * P
            s0 = ch * 512
            ps = psum.tile([P, 512], F32, tag=tg, bufs=bufs, name=f"op{sh}{blk}{ch}")
            nc.tensor.matmul(
                ps[:],
                wo_sb[:, c0 : c0 + P],
                heads_sb[:, s0 : s0 + 512],
                start=True,
                stop=True,
            )
            ot = ot_all[sh][:, blk, s0 : s0 + 512]
            if (blk + ch) % 2 == 0:
                nc.vector.tensor_scalar_add(ot, ps[:], bo_c[:, blk : blk + 1])
            else:
                nc.scalar.activation(
                    ot,
                    ps[:],
                    mybir.ActivationFunctionType.Identity,
                    bias=bo_c[:, blk : blk + 1],
                )

        # ---- attention passes ---------------------------------------------
        # p = (sh, hh): query-half sh, head hh. heads_sb per sh holds both
        # heads' outputs. The AV matmuls are emitted two t-tiles behind the
        # scores/exp so the PE pipelines through the cross-engine exp instead
        # of blocking on it. outproj(sh=0) units are interleaved into pass
        # 3's loop; filler matmuls cover pass 3's normalization so the PE
        # p-state stays hot for outproj(sh=1).
        heads = [
            headsp.tile([P, 1024], BF16, tag="heads", name=f"heads{sh}")
            for sh in range(2)
        ]
        norm_tail = [None]  # deferred expneg/broadcast/mult of the prior pass
        for p in range(4):
            sh, hh = p >> 1, p & 1
            h0 = sh * 1024
            hp = hh * 64
            av = psum.tile([65, 1024], F32, tag="av", bufs=1, name=f"av{p}")
            uts = {}

            def emit_av(tb, av=av, hh=hh, uts=uts):
                for n0 in (0, 512):
                    nc.tensor.matmul(
                        av[:, n0 : n0 + 512],
                        vaug_sb[:, tb, hh, :],
                        uts[tb][:, n0 : n0 + 512],
                        start=(tb == 0),
                        stop=(tb == 15),
                        skip_group_check=True,
                    )

            for tb in range(16):
                t0 = tb * P
                st = psum.tile([P, 1024], F32, tag="st", bufs=2, name=f"st{p}_{tb}")
                for n0 in (0, 512):
                    nc.tensor.matmul(
                        st[:, n0 : n0 + 512],
                        kt_sb[hp : hp + 64, t0 : t0 + P],
                        qt_sb[hp : hp + 64, h0 + n0 : h0 + n0 + 512],
                        start=True,
                        stop=True,
                    )
                ut = utp.tile([P, 1024], BF16, tag="ut", bufs=6)
                if tb in ACT_TBS:
                    nc.scalar.activation(
                        ut[:],
                        st[:],
                        mybir.ActivationFunctionType.Exp,
                        scale=0.125,
                        bias=act_bias[:],
                    )
                else:
                    nc.vector.tensor_scalar(
                        ut[:].bitcast(I16),
                        st[:],
                        SCH_A,
                        SCH_B,
                        mybir.AluOpType.mult,
                        mybir.AluOpType.add,
                    )
                uts[tb] = ut
                if tb == 2 and norm_tail[0] is not None:
                    # finish the previous pass's normalization now that this
                    # pass's first exps are in flight
                    norm_tail[0]()
                    norm_tail[0] = None
                if tb >= 4:
                    emit_av(tb - 4)
                if p == 3:
                    emit_op_unit(0, heads[0], tb // 2, tb % 2)
                    if tb == 7:
                        nc.sync.dma_start(out[:, 0:4096], ot_all[0][:, 0:4])
                    elif tb == 15:
                        nc.sync.dma_start(out[:, 4096:8192], ot_all[0][:, 4:8])
            for tb in (12, 13, 14, 15):
                emit_av(tb)
            # Normalization: recip of the denominator row is exp(-ln(d)) on
            # ACT (one act-table set, no DMA hops); GpSimd broadcasts it,
            # DVE multiplies into the bf16 heads tile. evac+ln run right
            # away (they release the AV psum bank); the rest is deferred
            # into the next pass (except on the last pass, where it all runs
            # now and filler matmuls keep the PE p-state hot).
            un = normp.tile([65, 1024], BF16, tag="un", bufs=2, name=f"un{p}")
            lnr = normp.tile([1, 1024], F32, tag="lnr", bufs=2, name=f"ln{p}")
            rcr = normp.tile([1, 1024], BF16, tag="rcr", bufs=2, name=f"rc{p}")
            def emit_evac(un=un, av=av):
                nc.scalar.activation(
                    un[:], av[0:65, :], mybir.ActivationFunctionType.Copy
                )

            def emit_ln(lnr=lnr, av=av):
                nc.scalar.activation(
                    lnr[:], av[64:65, :], mybir.ActivationFunctionType.Ln
                )

            def emit_expneg(rcr=rcr, lnr=lnr):
                nc.scalar.activation(
                    rcr[:], lnr[:], mybir.ActivationFunctionType.Exp, scale=-1.0
                )

            def emit_bcast_mult(p=p, sh=sh, hp=hp, un=un, rcr=rcr):
                rb = normp.tile([64, 1024], BF16, tag="rb", bufs=2, name=f"rb{p}")
                nc.gpsimd.partition_broadcast(rb[:], rcr[:], channels=64)
                nc.vector.tensor_tensor(
                    heads[sh][hp : hp + 64, :], un[0:64, :], rb[:],
                    mybir.AluOpType.mult,
                )

            if p < 3:
                emit_evac()
                emit_ln()

                def finish_norm(fns=(emit_expneg, emit_bcast_mult)):
                    for f in fns:
                        f()

                norm_tail[0] = finish_norm
                # Dependency-free pad matmuls so the PE pipeline never gaps
                # at the pass boundary (a gap resets the p-state ramp).
                for i in range(4):
                    wps = psum.tile(
                        [P, 512], F32, tag="op", bufs=2, name=f"pad{p}_{i}"
                    )
                    nc.tensor.matmul(
                        wps[:, 0:P], warm[:, 0:P], warm[:, 0:P],
                        start=True, stop=True,
                    )
            else:
                # Tail: pipeline the norm by s-halves so outproj(sh=1) ch=0
                # can start ~2us earlier. ln half 0 reads the psum row, half
                # 1 reads the evacuated copy (so the AV bank frees at evac).
                nc.scalar.activation(
                    lnr[:, 0:512], av[64:65, 0:512],
                    mybir.ActivationFunctionType.Ln,
                )
                nc.scalar.activation(
                    rcr[:, 0:512], lnr[:, 0:512],
                    mybir.ActivationFunctionType.Exp, scale=-1.0,
                )
                emit_evac()
                nc.scalar.activation(
                    lnr[:, 512:1024], av[64:65, 512:1024],
                    mybir.ActivationFunctionType.Ln,
                )
                nc.scalar.activation(
                    rcr[:, 512:1024], lnr[:, 512:1024],
                    mybir.ActivationFunctionType.Exp, scale=-1.0,
                )
                rb = normp.tile([64, 1024], BF16, tag="rb", bufs=2, name="rb3")
                for c0 in (0, 512):
                    nc.gpsimd.partition_broadcast(
                        rb[:, c0 : c0 + 512], rcr[:, c0 : c0 + 512], channels=64
                    )
                    nc.vector.tensor_tensor(
                        heads[sh][hp : hp + 64, c0 : c0 + 512],
                        un[0:64, c0 : c0 + 512],
                        rb[:, c0 : c0 + 512],
                        mybir.AluOpType.mult,
                    )
        for i in range(N_FILLER):
            wps = psum.tile([P, 512], F32, tag="op", bufs=2, name=f"fill{i}")
            nc.tensor.matmul(
                wps[:], warm[:, 0:P], warm[:], start=True, stop=True
            )
        op1_rot = [("op", 2), ("op", 2), ("st", 2), ("st", 2), ("av", 1)]
        for i in range(16):
            ch, blk = i // 8, i % 8
            tg, bfs = op1_rot[i % 5]
            emit_op_unit(1, heads[1], blk, ch, tg=tg, bufs=bfs)
            if i == 11:
                nc.sync.dma_start(out[:, 8192:12288], ot_all[1][:, 0:4])
            elif i == 15:
                nc.sync.dma_start(out[:, 12288:16384], ot_all[1][:, 4:8])

    nc.finalize()
    return nc


_NC_CACHE = None


def _get_nc():
    global _NC_CACHE
    if _NC_CACHE is None:
        _NC_CACHE = _build_nc()
    return _NC_CACHE


def _make_in_maps(embeddings, Wq, bq, Wk, bk, Wv, bv, Wo, bo):
    import ml_dtypes

    bf16 = np.dtype(ml_dtypes.bfloat16)
    # [sc, pi, dc, sl]: et4[sc, pi, dc*512+sl] = emb.T[dc*128+pi, sc*512+sl]
    embT = np.ascontiguousarray(embeddings.T.astype(bf16))  # [1024, 2048]
    et4 = np.ascontiguousarray(
        embT.reshape(8, P, 4, 512).transpose(2, 1, 0, 3).reshape(4, P, 4096)
    )
    in_maps = []
    for c in range(NCORES):
        hs = [2 * c, 2 * c + 1]
        wqkv_cat = np.concatenate(
            [Wq[hs[0]], Wq[hs[1]], Wk[hs[0]], Wk[hs[1]], Wv[hs[0]], Wv[hs[1]]],
            axis=1,
        ).astype(bf16)  # [1024, 384] = d x (q|k|v)
        # -> [pi, grp*8*128 + dc*128 + c]: one contiguous 6KB run/partition
        wqkv = np.ascontiguousarray(
            wqkv_cat.reshape(8, P, 3, P).transpose(1, 2, 0, 3).reshape(P, 3072)
        )
        bqk = np.stack(
            [np.concatenate([bq[hs[0]], bq[hs[1]]]),
             np.concatenate([bk[hs[0]], bk[hs[1]]])],
            axis=1,
        ).astype(np.float32)  # [128, 2]
        # Fold the V bias through attention (softmax weights sum to 1) and
        # the output projection: + Wo_rows^T bv_rows, done host-side.
        bv_rows = np.concatenate([bv[hs[0]], bv[hs[1]]])  # [128]
        bo_eff = Wo[c * P : (c + 1) * P].T.astype(np.float64) @ bv_rows.astype(
            np.float64
        )
        if c == 0:
            bo_eff = bo_eff + bo
        in_maps.append(
            {
                "et": et4,
                "wqkv": np.ascontiguousarray(wqkv),
                "bqk": np.ascontiguousarray(bqk),
                "bo": np.ascontiguousarray(
                    bo_eff.reshape(8, P).T, dtype=np.float32
                ),
                "wo": np.ascontiguousarray(Wo[c * P : (c + 1) * P].astype(bf16)),
            }
        )
    return in_maps


def kernel(embeddings, Wq, bq, Wk, bk, Wv, bv, Wo, bo, **run_kwargs):
    """Full-input / full-output MHA. Shards across 8 NeuronCores internally."""
    nc = _get_nc()
    in_maps = _make_in_maps(
        np.asarray(embeddings, np.float32),
        np.asarray(Wq, np.float32),
        np.asarray(bq, np.float32),
        np.asarray(Wk, np.float32),
        np.asarray(bk, np.float32),
        np.asarray(Wv, np.float32),
        np.asarray(bv, np.float32),
        np.asarray(Wo, np.float32),
        np.asarray(bo, np.float32),
    )
    res = run_bass_kernel_spmd(nc, in_maps, list(range(NCORES)), **run_kwargs)
    return _unshard(res.results)


def _unshard(results):
    # Row-parallel output projection: sum per-core bf16 partials in fp32,
    # then undo the on-chip [pi, sh, blk, sl] layout (c = blk*128+pi,
    # s = sh*1024+sl).
    acc = results[0]["out"].astype(np.float32)
    for r_ in results[1:]:
        acc += r_["out"].astype(np.float32)
    acc = acc.reshape(P, 2, 8, 1024).transpose(2, 0, 1, 3).reshape(D, S)
    return np.ascontiguousarray(acc.T)


if __name__ == "__main__":
    rng = np.random.default_rng(0)
    emb = rng.standard_normal((S, D), dtype=np.float32)
    mk = lambda *sh: (rng.standard_normal(sh, dtype=np.float32) * 0.02)
    o = kernel(
        embeddings=emb,
        Wq=mk(H, D, DK), bq=mk(H, DK),
        Wk=mk(H, D, DK), bk=mk(H, DK),
        Wv=mk(H, D, DK), bv=mk(H, DK),
        Wo=mk(H * DK, D), bo=mk(D),
    )
    print(o.shape, o.dtype)


# revision 60
# speedup vs baseline: 1.0192x; 1.0192x over previous
"""Trainium2 Bass kernel: 16-head MHA (S=2048, D=1024, Dk=Dv=64) on 8 NeuronCores.

Sharding: tensor-parallel over heads - 2 heads per core. Each core projects
Q/K/V for its 2 heads, computes scores in transposed layout S^T[t, s] =
K_h Q_h^T, exponentiates with the 1/sqrt(64) scale fused in, and accumulates
heads^T = V_aug^T @ exp(S^T) with a ones-column so the softmax denominator
falls out of the same matmul (PSUM row 64).

Differences vs the naive schedule, all aimed at keeping the PE tensor engine
busy at full clock end to end:
  - PE warm-up matmuls run during the initial DMA wait so the p-state ramps
    to full frequency before the first real matmul.
  - Embeddings stream in four 1MB chunks (host-prearranged to contiguous
    per-chunk layout) and QKV compute starts as soon as chunk 0 lands.
  - The softmax exp is split across engines: most tiles use the exact Exp on
    the Scalar/ACT engine (output scaled by the mean Schraudolph factor so
    the two tile families agree), the rest use a Schraudolph-style bit-trick
    exp on the Vector engine (affine op + fp32->int16 convert, int16 bits
    reinterpreted as bf16).
  - The V projection bias is folded into the per-core output bias on the
    host (softmax weights sum to one, so +bv passes through attention and
    lands in Wo^T bv).
  - Softmax normalization: ACT evacuates the AV psum, DVE takes the
    reciprocal of the denominator row, GpSimd broadcasts it and applies the
    multiply (all off the critical engines).
  - The sh=0 output projection is scheduled between attention passes 2 and 3
    and filler matmuls run while the last softmax normalization completes,
    so the PE never idles long enough to drop its p-state before the final
    output projection.
  - Output partials ship as bf16 (summed in fp32 on the host), halving the
    output drain, and all output-DMA triggers ride the otherwise-idle sync
    sequencer.

Matmuls run in bf16 with fp32 PSUM accumulation.
"""

import math

import numpy as np

import concourse.tile as tile_mod
from concourse import bacc, mybir
from concourse.bass_utils import run_bass_kernel_spmd
from concourse.vector_clock import ScopedClock, VectorClock

F32 = mybir.dt.float32
BF16 = mybir.dt.bfloat16
I16 = mybir.dt.int16

S, D, H, DK = 2048, 1024, 16, 64
P = 128
NCORES = 8

# Schraudolph bf16 exp: u = bitcast_bf16(int16(s * SCH_A + SCH_B)) ~= exp(s/8)
SCH_A = 0.125 * 128.0 / math.log(2.0)
SCH_B = 127.0 * 128.0
# The bit-trick overestimates exp by x1.0418 on average; scale the exact-exp
# tiles by the same factor (bias adds ln inside the exponent) so both tile
# families are consistently biased and the bias cancels in the softmax.
ACT_BIAS = 0.0399
# Per-pass split of the 16 t-tiles: 8 on ACT (exact), 8 on DVE (bit trick).
# tb 0/1 go to DVE so a new pass never waits on ACT finishing the previous
# pass's evac+ln.
ACT_TBS = frozenset([1, 2, 4, 6, 8, 10, 12, 14])

N_WARMUP = 22
N_FILLER = 10


def _patched_drain_and_barrier(self, tick_clock, wait_clock):
    """This container's walrus build caps CTRL-type instructions at one sem
    wait, but Tile's exit drain carries one wait per outstanding proc. Emit
    one Drain per outstanding proc instead, each with a single wait."""
    gc = tick_clock.global_clock
    vec = list(gc)
    for i, t in enumerate(vec):
        if t <= 0:
            continue
        pv = [0] * len(vec)
        pv[i] = t
        d = self.nc.sync.drain()
        wait_clock.add_sem_waits(d.ins, ScopedClock({None: VectorClock(pv)}))

    self.nc.all_engine_barrier()
    assert self.sems is not None
    popped = self.nc._tile_sem_poison_stack.pop()
    assert popped is self._sem_poison
    self.nc.clear_and_free_semaphores(list(self.sems.allocated().values()))
    self.nc.all_engine_barrier()


tile_mod.TileContext._drain_and_barrier = _patched_drain_and_barrier


def _pin_act_tables():
    """Pin ACT-table selection to natural_log_exp_and_others (covers Exp,
    Ln, Copy, Identity) so the per-pass Exp<->Ln mix doesn't thrash
    ACT_TABLE_LOADs. Other sets are emptied but keep their positions so the
    emitted act_func_set_id still indexes act_info.json correctly."""
    import concourse.bacc as bacc_mod

    orig = bacc_mod.get_activation_tables
    keep = "natural_log_exp_and_others"

    def pinned(arch, _orig=orig, _keep=keep):
        t = dict(_orig(arch))
        if _keep not in t:
            return t
        return {n: (f if n == _keep else set()) for n, f in t.items()}

    bacc_mod.get_activation_tables = pinned


_pin_act_tables()


def _build_nc():
    from contextlib import ExitStack

    tile = tile_mod
    nc = bacc.Bacc(None)

    # et: embeddings^T prearranged on host as [sc, pi, dc*512+sl] so each
    # 1MB s-chunk is one contiguous DMA with 8KB runs per partition.
    # wqkv: host-prearranged as [grp, pi, dc*128+c] (grp = q/k/v) so each
    # projection's weights are one contiguous transfer.
    # out: [pi, sh*8192 + blk*1024 + sl] so each s-half ships as one DMA
    # with 16KB contiguous runs per partition (128 descriptors instead of
    # 2048 1KB ones).
    et = nc.declare_dram_parameter("et", [4, P, 4096], BF16, isOutput=False)
    wqkv = nc.declare_dram_parameter("wqkv", [P, 3072], BF16, isOutput=False)
    bqk = nc.declare_dram_parameter("bqk", [P, 2], F32, isOutput=False)
    bo = nc.declare_dram_parameter("bo", [P, 8], F32, isOutput=False)
    wo = nc.declare_dram_parameter("wo", [P, D], BF16, isOutput=False)
    out = nc.declare_dram_parameter("out", [P, 16384], BF16, isOutput=True)

    with tile.TileContext(nc) as tc, ExitStack() as ctx:
        consts = ctx.enter_context(tc.tile_pool(name="consts", bufs=1))
        qkv = ctx.enter_context(tc.tile_pool(name="qkv", bufs=1))
        utp = ctx.enter_context(tc.tile_pool(name="ut", bufs=3))
        headsp = ctx.enter_context(tc.tile_pool(name="heads", bufs=2))
        normp = ctx.enter_context(tc.tile_pool(name="norm", bufs=4))
        outp = ctx.enter_context(tc.tile_pool(name="outp", bufs=3))
        psum = ctx.enter_context(tc.tile_pool(name="psum", bufs=1, space="PSUM"))

        # ---- input DMAs ----------------------------------------------------
        # Small consts first, then wqkv split across both HWDGE rings, then
        # the four et chunks (each split across rings), wo last.
        bqk_sb = consts.tile([P, 2], F32)
        nc.sync.dma_start(bqk_sb[:], bqk[:])
        bo_c = consts.tile([P, 8], F32)
        nc.scalar.dma_start(bo_c[:], bo[:])
        wqkv_sb = consts.tile([P, 3, 8, P], BF16)  # [pi, grp, dc, c]
        nc.sync.dma_start(wqkv_sb[:, 0], wqkv[:, 0:1024])
        nc.scalar.dma_start(wqkv_sb[:, 1:3], wqkv[:, 1024:3072])
        et_sc = []
        for sc in range(4):
            t = consts.tile([P, 8, 512], BF16, name=f"et{sc}")
            et_sc.append(t)
            eng = nc.scalar if sc % 2 else nc.sync
            eng.dma_start(t[:], et[sc])
        wo_sb = consts.tile([P, D], BF16)
        nc.scalar.dma_start(wo_sb[:], wo[:])

        # ---- SBUF compute tiles -------------------------------------------
        warm = consts.tile([P, 512], BF16)
        nc.vector.memset(warm[:], 0.25)
        act_bias = consts.tile([P, 1], F32)
        nc.vector.memset(act_bias[:], ACT_BIAS)
        qt_sb = qkv.tile([P, S], BF16)
        kt_sb = qkv.tile([P, S], BF16)
        # vaug[t-part, tb, head, 0:64]=v, [*, 64]=1 (softmax denominator row)
        vaug_sb = qkv.tile([P, 16, 2, 65], BF16)
        nc.vector.memset(vaug_sb[:, :, :, 64:65], 1.0)

        # PSUM: st [128,1024]x2 (4 banks), av [65,1024]x1 (2), op [128,512]x2
        # (2) = 8 banks. QKV-phase psums rotate over the same slots.
        mm_bufs = {"st": 2, "av": 1, "op": 2}
        rot_state = [0]
        rot_cycle = ["op", "op", "st", "st", "av"]

        def next_tag():
            t = rot_cycle[rot_state[0] % 5]
            rot_state[0] += 1
            return t

        # ---- PE warm-up ----------------------------------------------------
        # Dead matmuls during the DMA wait ramp the PE p-state to full clock.
        for i in range(N_WARMUP):
            wps = psum.tile([P, 512], F32, tag="op", bufs=2, name=f"warm{i}")
            nc.tensor.matmul(
                wps[:], warm[:, 0:P], warm[:], start=True, stop=True
            )

        # ---- QKV projections, one et chunk at a time -----------------------
        for sc in range(4):
            s0 = sc * 512
            for which, dst in ((0, qt_sb), (1, kt_sb)):
                tg = next_tag()
                ps = psum.tile(
                    [P, 512], F32, tag=tg, bufs=mm_bufs[tg], name=f"qk{sc}{which}"
                )
                for dc in range(8):
                    nc.tensor.matmul(
                        ps[:],
                        wqkv_sb[:, which, dc, :],
                        et_sc[sc][:, dc, :],
                        start=(dc == 0),
                        stop=(dc == 7),
                    )
                nc.vector.tensor_scalar_add(
                    dst[:, s0 : s0 + 512], ps[:], bqk_sb[:, which : which + 1]
                )
            for tl in range(4):
                tb = sc * 4 + tl
                tg = next_tag()
                ps = psum.tile(
                    [P, P], F32, tag=tg, bufs=mm_bufs[tg], name=f"v{tb}"
                )
                for dc in range(8):
                    nc.tensor.matmul(
                        ps[:],
                        et_sc[sc][:, dc, tl * P : tl * P + P],
                        wqkv_sb[:, 2, dc, :],
                        start=(dc == 0),
                        stop=(dc == 7),
                    )
                # v psum cols [0:64]=head0, [64:128]=head1 -> vaug (no bias:
                # bv is folded into bo on the host)
                nc.scalar.activation(
                    vaug_sb[:, tb, :, 0:64],
                    ps[:].rearrange("p (hh v) -> p hh v", hh=2),
                    mybir.ActivationFunctionType.Copy,
                )

        # Per-s-half output accumulators: all 16 units' results collect here
        # and ship as ONE 2MB DMA with 16KB contiguous runs per partition.
        ot_all = [
            outp.tile([P, 8, 1024], BF16, tag=f"ot{sh}", bufs=1, name=f"ot{sh}")
            for sh in range(2)
        ]

        def emit_op_unit(sh, heads_sb, blk, ch, tg="op", bufs=2):
            # out^T[c, s] = wo_rows.T @ heads^T (+ bo as per-partition scalar)
            c0 = blk * P
            s0 = ch * 512
            ps = psum.tile([P, 512], F32, tag=tg, bufs=bufs, name=f"op{sh}{blk}{ch}")
            nc.tensor.matmul(
                ps[:],
                wo_sb[:, c0 : c0 + P],
                heads_sb[:, s0 : s0 + 512],
                start=True,
                stop=True,
            )
            ot = ot_all[sh][:, blk, s0 : s0 + 512]
            if (blk + ch) % 2 == 0:
                nc.vector.tensor_scalar_add(ot, ps[:], bo_c[:, blk : blk + 1])
            else:
                nc.scalar.activation(
                    ot,
                    ps[:],
                    mybir.ActivationFunctionType.Identity,
                    bias=bo_c[:, blk : blk + 1],
                )

        # ---- attention passes ---------------------------------------------
        # p = (sh, hh): query-half sh, head hh. heads_sb per sh holds both
        # heads' outputs. The AV matmuls are emitted two t-tiles behind the
        # scores/exp so the PE pipelines through the cross-engine exp instead
        # of blocking on it. outproj(sh=0) units are interleaved into pass
        # 3's loop; filler matmuls cover pass 3's normalization so the PE
        # p-state stays hot for outproj(sh=1).
        heads = [
            headsp.tile([P, 1024], BF16, tag="heads", name=f"heads{sh}")
            for sh in range(2)
        ]
        norm_tail = [None]  # deferred expneg/broadcast/mult of the prior pass
        for p in range(4):
            sh, hh = p >> 1, p & 1
            h0 = sh * 1024
            hp = hh * 64
            av = psum.tile([65, 1024], F32, tag="av", bufs=1, name=f"av{p}")
            uts = {}

            def emit_av(tb, av=av, hh=hh, uts=uts):
                for n0 in (0, 512):
                    nc.tensor.matmul(
                        av[:, n0 : n0 + 512],
                        vaug_sb[:, tb, hh, :],
                        uts[tb][:, n0 : n0 + 512],
                        start=(tb == 0),
                        stop=(tb == 15),
                        skip_group_check=True,
                    )

            for tb in range(16):
                t0 = tb * P
                st = psum.tile([P, 1024], F32, tag="st", bufs=2, name=f"st{p}_{tb}")
                for n0 in (0, 512):
                    nc.tensor.matmul(
                        st[:, n0 : n0 + 512],
                        kt_sb[hp : hp + 64, t0 : t0 + P],
                        qt_sb[hp : hp + 64, h0 + n0 : h0 + n0 + 512],
                        start=True,
                        stop=True,
                    )
                ut = utp.tile([P, 1024], BF16, tag="ut", bufs=6)
                if tb in ACT_TBS:
                    nc.scalar.activation(
                        ut[:],
                        st[:],
                        mybir.ActivationFunctionType.Exp,
                        scale=0.125,
                        bias=act_bias[:],
                    )
                else:
                    nc.vector.tensor_scalar(
                        ut[:].bitcast(I16),
                        st[:],
                        SCH_A,
                        SCH_B,
                        mybir.AluOpType.mult,
                        mybir.AluOpType.add,
                    )
                uts[tb] = ut
                if tb == 2 and norm_tail[0] is not None:
                    # finish the previous pass's normalization now that this
                    # pass's first exps are in flight
                    norm_tail[0]()
                    norm_tail[0] = None
                if tb >= 4:
                    emit_av(tb - 4)
                if p == 3:
                    emit_op_unit(0, heads[0], tb // 2, tb % 2)
                    if tb == 7:
                        nc.sync.dma_start(out[:, 0:4096], ot_all[0][:, 0:4])
                    elif tb == 15:
                        nc.sync.dma_start(out[:, 4096:8192], ot_all[0][:, 4:8])
            for tb in (12, 13, 14, 15):
                emit_av(tb)
            # Normalization: recip of the denominator row is exp(-ln(d)) on
            # ACT (one act-table set, no DMA hops); GpSimd broadcasts it,
            # DVE multiplies into the bf16 heads tile. evac+ln run right
            # away (they release the AV psum bank); the rest is deferred
            # into the next pass (except on the last pass, where it all runs
            # now and filler matmuls keep the PE p-state hot).
            un = normp.tile([65, 1024], BF16, tag="un", bufs=2, name=f"un{p}")
            lnr = normp.tile([1, 1024], F32, tag="lnr", bufs=2, name=f"ln{p}")
            rcr = normp.tile([1, 1024], BF16, tag="rcr", bufs=2, name=f"rc{p}")
            def emit_evac(un=un, av=av):
                nc.scalar.activation(
                    un[:], av[0:65, :], mybir.ActivationFunctionType.Copy
                )

            def emit_ln(lnr=lnr, av=av):
                nc.scalar.activation(
                    lnr[:], av[64:65, :], mybir.ActivationFunctionType.Ln
                )

            def emit_expneg(rcr=rcr, lnr=lnr):
                nc.scalar.activation(
                    rcr[:], lnr[:], mybir.ActivationFunctionType.Exp, scale=-1.0
                )

            def emit_bcast_mult(p=p, sh=sh, hp=hp, un=un, rcr=rcr):
                rb = normp.tile([64, 1024], BF16, tag="rb", bufs=2, name=f"rb{p}")
                nc.gpsimd.partition_broadcast(rb[:], rcr[:], channels=64)
                nc.vector.tensor_tensor(
                    heads[sh][hp : hp + 64, :], un[0:64, :], rb[:],
                    mybir.AluOpType.mult,
                )

            if p < 3:
                emit_evac()
                emit_ln()

                def finish_norm(fns=(emit_expneg, emit_bcast_mult)):
                    for f in fns:
                        f()

                norm_tail[0] = finish_norm
                # Dependency-free pad matmuls so the PE pipeline never gaps
                # at the pass boundary (a gap resets the p-state ramp).
                for i in range(4):
                    wps = psum.tile(
                        [P, 512], F32, tag="op", bufs=2, name=f"pad{p}_{i}"
                    )
                    nc.tensor.matmul(
                        wps[:, 0:P], warm[:, 0:P], warm[:, 0:P],
                        start=True, stop=True,
                    )
            else:
                # Tail: pipeline the norm by s-halves so outproj(sh=1) ch=0
                # can start ~2us earlier. ln half 0 reads the psum row, half
                # 1 reads the evacuated copy (so the AV bank frees at evac).
                nc.scalar.activation(
                    lnr[:, 0:512], av[64:65, 0:512],
                    mybir.ActivationFunctionType.Ln,
                )
                nc.scalar.activation(
                    rcr[:, 0:512], lnr[:, 0:512],
                    mybir.ActivationFunctionType.Exp, scale=-1.0,
                )
                emit_evac()
                nc.scalar.activation(
                    lnr[:, 512:1024], av[64:65, 512:1024],
                    mybir.ActivationFunctionType.Ln,
                )
                nc.scalar.activation(
                    rcr[:, 512:1024], lnr[:, 512:1024],
                    mybir.ActivationFunctionType.Exp, scale=-1.0,
                )
                rb = normp.tile([64, 1024], BF16, tag="rb", bufs=2, name="rb3")
                for c0 in (0, 512):
                    nc.gpsimd.partition_broadcast(
                        rb[:, c0 : c0 + 512], rcr[:, c0 : c0 + 512], channels=64
                    )
                    nc.vector.tensor_tensor(
                        heads[sh][hp : hp + 64, c0 : c0 + 512],
                        un[0:64, c0 : c0 + 512],
                        rb[:, c0 : c0 + 512],
                        mybir.AluOpType.mult,
                    )
        for i in range(N_FILLER):
            wps = psum.tile([P, 512], F32, tag="op", bufs=2, name=f"fill{i}")
            nc.tensor.matmul(
                wps[:], warm[:, 0:P], warm[:], start=True, stop=True
            )
        op1_rot = [("op", 2), ("op", 2), ("st", 2), ("st", 2), ("av", 1)]
        for i in range(16):
            ch, blk = i // 8, i % 8
            tg, bfs = op1_rot[i % 5]
            emit_op_unit(1, heads[1], blk, ch, tg=tg, bufs=bfs)
            if i == 11:
                nc.sync.dma_start(out[:, 8192:12288], ot_all[1][:, 0:4])
            elif i == 15:
                nc.sync.dma_start(out[:, 12288:16384], ot_all[1][:, 4:8])

    nc.finalize()
    return nc


_NC_CACHE = None


def _get_nc():
    global _NC_CACHE
    if _NC_CACHE is None:
        _NC_CACHE = _build_nc()
    return _NC_CACHE


def _make_in_maps(embeddings, Wq, bq, Wk, bk, Wv, bv, Wo, bo):
    import ml_dtypes

    bf16 = np.dtype(ml_dtypes.bfloat16)
    # [sc, pi, dc, sl]: et4[sc, pi, dc*512+sl] = emb.T[dc*128+pi, sc*512+sl]
    embT = np.ascontiguousarray(embeddings.T.astype(bf16))  # [1024, 2048]
    et4 = np.ascontiguousarray(
        embT.reshape(8, P, 4, 512).transpose(2, 1, 0, 3).reshape(4, P, 4096)
    )
    in_maps = []
    for c in range(NCORES):
        hs = [2 * c, 2 * c + 1]
        wqkv_cat = np.concatenate(
            [Wq[hs[0]], Wq[hs[1]], Wk[hs[0]], Wk[hs[1]], Wv[hs[0]], Wv[hs[1]]],
            axis=1,
        ).astype(bf16)  # [1024, 384] = d x (q|k|v)
        # -> [pi, grp*8*128 + dc*128 + c]: one contiguous 6KB run/partition
        wqkv = np.ascontiguousarray(
            wqkv_cat.reshape(8, P, 3, P).transpose(1, 2, 0, 3).reshape(P, 3072)
        )
        bqk = np.stack(
            [np.concatenate([bq[hs[0]], bq[hs[1]]]),
             np.concatenate([bk[hs[0]], bk[hs[1]]])],
            axis=1,
        ).astype(np.float32)  # [128, 2]
        # Fold the V bias through attention (softmax weights sum to 1) and
        # the output projection: + Wo_rows^T bv_rows, done host-side.
        bv_rows = np.concatenate([bv[hs[0]], bv[hs[1]]])  # [128]
        bo_eff = Wo[c * P : (c + 1) * P].T.astype(np.float64) @ bv_rows.astype(
            np.float64
        )
        if c == 0:
            bo_eff = bo_eff + bo
        in_maps.append(
            {
                "et": et4,
                "wqkv": np.ascontiguousarray(wqkv),
                "bqk": np.ascontiguousarray(bqk),
                "bo": np.ascontiguousarray(
                    bo_eff.reshape(8, P).T, dtype=np.float32
                ),
                "wo": np.ascontiguousarray(Wo[c * P : (c + 1) * P].astype(bf16)),
            }
        )
    return in_maps


def kernel(embeddings, Wq, bq, Wk, bk, Wv, bv, Wo, bo, **run_kwargs):
    """Full-input / full-output MHA. Shards across 8 NeuronCores internally."""
    nc = _get_nc()
    in_maps = _make_in_maps(
        np.asarray(embeddings, np.float32),
        np.asarray(Wq, np.float32),
        np.asarray(bq, np.float32),
        np.asarray(Wk, np.float32),
        np.asarray(bk, np.float32),
        np.asarray(Wv, np.float32),
        np.asarray(bv, np.float32),
        np.asarray(Wo, np.float32),
        np.asarray(bo, np.float32),
    )
    res = run_bass_kernel_spmd(nc, in_maps, list(range(NCORES)), **run_kwargs)
    return _unshard(res.results)


def _unshard(results):
    # Row-parallel output projection: sum per-core bf16 partials in fp32,
    # then undo the on-chip [pi, sh, blk, sl] layout (c = blk*128+pi,
    # s = sh*1024+sl).
    acc = results[0]["out"].astype(np.float32)
    for r_ in results[1:]:
        acc += r_["out"].astype(np.float32)
    acc = acc.reshape(P, 2, 8, 1024).transpose(2, 0, 1, 3).reshape(D, S)
    return np.ascontiguousarray(acc.T)


if __name__ == "__main__":
    rng = np.random.default_rng(0)
    emb = rng.standard_normal((S, D), dtype=np.float32)
    mk = lambda *sh: (rng.standard_normal(sh, dtype=np.float32) * 0.02)
    o = kernel(
        embeddings=emb,
        Wq=mk(H, D, DK), bq=mk(H, DK),
        Wk=mk(H, D, DK), bk=mk(H, DK),
        Wv=mk(H, D, DK), bv=mk(H, DK),
        Wo=mk(H * DK, D), bo=mk(D),
    )
    print(o.shape, o.dtype)
